# revision 28
# baseline (speedup 1.0000x reference)
"""Trainium2 Bass kernel: batched CRF Viterbi decode.

Problem: x [1024, 1024, 41] f32 emissions + tiny transition params ->
best tag sequence [1024, 1024] int32 (torchcrf CRF.decode semantics).

Strategy: data-parallel over batch across 8 NeuronCores (128 batches/core
= 128 SBUF partitions). Each core runs the sequential Viterbi scan over
T=1024 steps fully on-chip:

  forward (per step, DVE):
    tmp[b, j, i]  = T'[j, i] + s[b, i]           (T' = transitions^T, replicated)
    tmp2[b, j, i] = tmp + e_t[b, j]              (exact reference add order)
    s'[b, j]      = max_i tmp2                   (reduce X; next score, incl. e)
    eq            = (tmp2 == s')                 (broadcast s' over i; bf16 out)
    cand          = eq * -64 + (64 + i)          (fused scalar_tensor_tensor, bf16)
    bp[b, j]      = min_i cand  -> u16 SBUF      (first-index argmax, == jnp.argmax)

  The value path (tmp/tmp2/s') is fp32 and replicates the reference's
  rounding order bitwise, so decoded tags match jnp exactly (verified
  0/1048576 on hardware). The bp path runs in bf16 — eq is 0/1 and cand
  holds ints <= 104, both bf16-exact — which gives the scalar_tensor_tensor
  full rate (two non-bf16 SBUF sources would halve it) and the all-2-byte
  reduce_min the DVE 2x perf mode.

  backtrace (per step): one fused scalar_tensor_tensor computes
  (iota == tag) * bp with accum_out = sum = bp[tag].

Backpointers (1023*41 u16/partition = 10.7 MB) live entirely in SBUF.
Emissions stream in 64-step chunks, double-buffered.
"""

import numpy as np

import concourse.bacc as bacc
import concourse.mybir as mybir
from concourse import bass_utils
from concourse.tile import TileContext

B_FULL = 1024
T_FULL = 1024
C = 41
CC = C * C
N_CORES = 8
P = B_FULL // N_CORES  # 128 batches per core == SBUF partitions
BIG = 64.0  # offset for the argmax iota trick (bf16-exact: all values <= 104)
TCHUNK = 64  # emission timesteps per DMA chunk

f32 = mybir.dt.float32
bf16 = mybir.dt.bfloat16
i32 = mybir.dt.int32
u32 = mybir.dt.uint32
u8 = mybir.dt.uint8
Alu = mybir.AluOpType
AxX = mybir.AxisListType.X


def build_viterbi_nc(T: int = T_FULL):
    nc = bacc.Bacc("TRN2", target_bir_lowering=False, debug=False, num_devices=N_CORES)
    x = nc.dram_tensor("x", [P, T, C], f32, kind="ExternalInput")
    t_T = nc.dram_tensor("t_T", [P, CC], f32, kind="ExternalInput")
    iota_big = nc.dram_tensor("iota_big", [P, CC], bf16, kind="ExternalInput")
    iota_row = nc.dram_tensor("iota_row", [P, C], f32, kind="ExternalInput")
    start_rep = nc.dram_tensor("start_rep", [P, C], f32, kind="ExternalInput")
    end_rep = nc.dram_tensor("end_rep", [P, C], f32, kind="ExternalInput")
    tags = nc.dram_tensor("tags", [P, T], i32, kind="ExternalOutput")
    with TileContext(nc) as tc:
        _viterbi_body(nc, tc, x, t_T, iota_big, iota_row, start_rep, end_rep, tags, T)
    nc.compile()
    return nc


def _viterbi_body(nc, tc, x, t_T, iota_big, iota_row, start_rep, end_rep, tags, T):
    with (
        tc.tile_pool(name="const", bufs=1) as cpool,
        tc.tile_pool(name="big", bufs=1) as bpool,
        tc.tile_pool(name="emis", bufs=2) as epool,
        tc.tile_pool(name="work", bufs=2) as wpool,
        tc.tile_pool(name="bp_work", bufs=3) as bp_pool,
        tc.tile_pool(name="small", bufs=3) as spool,
    ):
        Trep = cpool.tile([P, CC], f32, tag="Trep")
        nc.sync.dma_start(out=Trep[:, :], in_=t_T[:, :])
        IOTB = cpool.tile([P, CC], bf16, tag="IOTB")
        nc.sync.dma_start(out=IOTB[:, :], in_=iota_big[:, :])
        IOTR = cpool.tile([P, C], f32, tag="IOTR")
        nc.sync.dma_start(out=IOTR[:, :], in_=iota_row[:, :])
        SREP = cpool.tile([P, C], f32, tag="SREP")
        nc.sync.dma_start(out=SREP[:, :], in_=start_rep[:, :])
        EREP = cpool.tile([P, C], f32, tag="EREP")
        nc.sync.dma_start(out=EREP[:, :], in_=end_rep[:, :])

        BP = bpool.tile([P, (T - 1) * C], mybir.dt.uint16, tag="BP")
        TAGF = bpool.tile([P, T], f32, tag="TAGF")

        Trep3 = Trep[:, :].rearrange("p (j i) -> p j i", i=C)

        s = None
        e_tile = None
        for t in range(T):
            if t % TCHUNK == 0:
                n_steps = min(TCHUNK, T - t)
                e_tile = epool.tile([P, TCHUNK * C], f32, tag="e")
                nc.sync.dma_start(
                    out=e_tile[:, 0 : n_steps * C].rearrange("p (a c) -> p a c", c=C),
                    in_=x[:, t : t + n_steps, :],
                )
            ecol = e_tile[:, (t % TCHUNK) * C : ((t % TCHUNK) + 1) * C]
            s_new = spool.tile([P, C], f32, tag="s")
            if t == 0:
                nc.vector.tensor_tensor(
                    out=s_new[:, :], in0=SREP[:, :], in1=ecol, op=Alu.add
                )
            else:
                # tmp = T'[j,i] + s[b,i] ; tmp2 = tmp + e[b,j]  (exact ref order)
                tmp = wpool.tile([P, CC], f32, tag="tmp")
                tmp3 = tmp[:, :].rearrange("p (j i) -> p j i", i=C)
                nc.vector.tensor_tensor(
                    out=tmp3,
                    in0=Trep3,
                    in1=s[:, :].unsqueeze(1).broadcast_to([P, C, C]),
                    op=Alu.add,
                )
                tmp2 = wpool.tile([P, CC], f32, tag="tmp2")
                tmp23 = tmp2[:, :].rearrange("p (j i) -> p j i", i=C)
                nc.vector.tensor_tensor(
                    out=tmp23,
                    in0=tmp3,
                    in1=ecol.unsqueeze(2).broadcast_to([P, C, C]),
                    op=Alu.add,
                )
                # s_new[b,j] = max_i tmp2  (== reference next score, incl. e)
                nc.vector.tensor_reduce(
                    out=s_new[:, :], in_=tmp23, axis=AxX, op=Alu.max
                )
                # backpointer path (off critical chain, bf16 for DVE 2x modes)
                eq = bp_pool.tile([P, CC], bf16, tag="eq")
                nc.vector.tensor_tensor(
                    out=eq[:, :].rearrange("p (j i) -> p j i", i=C),
                    in0=tmp23,
                    in1=s_new[:, :].unsqueeze(2).broadcast_to([P, C, C]),
                    op=Alu.is_equal,
                )
                cand = bp_pool.tile([P, CC], bf16, tag="cand")
                nc.vector.scalar_tensor_tensor(
                    out=cand[:, :],
                    in0=eq[:, :],
                    scalar=-BIG,
                    in1=IOTB[:, :],
                    op0=Alu.mult,
                    op1=Alu.add,
                )
                nc.vector.tensor_reduce(
                    out=BP[:, (t - 1) * C : t * C],
                    in_=cand[:, :].rearrange("p (j i) -> p j i", i=C),
                    axis=AxX,
                    op=Alu.min,
                )
            s = s_new

        fin = spool.tile([P, C], f32, tag="fin")
        nc.vector.tensor_tensor(out=fin[:, :], in0=s[:, :], in1=EREP[:, :], op=Alu.add)
        mx8 = spool.tile([P, 8], f32, tag="mx8")
        nc.vector.max(out=mx8[:, :], in_=fin[:, :])
        idx8 = spool.tile([P, 8], u32, tag="idx8")
        nc.vector.max_index(out=idx8[:, :], in_max=mx8[:, :], in_values=fin[:, :])
        nc.vector.tensor_copy(out=TAGF[:, T - 1 : T], in_=idx8[:, 0:1])
        for t in range(T - 1, 0, -1):
            # fused: out = (iota == tag) * bp ; accum_out = sum(out) = bp[tag]
            oh = spool.tile([P, C], f32, tag="oh")
            nc.vector.scalar_tensor_tensor(
                out=oh[:, :],
                in0=IOTR[:, :],
                scalar=TAGF[:, t : t + 1],
                in1=BP[:, (t - 1) * C : t * C],
                op0=Alu.is_equal,
                op1=Alu.mult,
                accum_out=TAGF[:, t - 1 : t],
            )
        TAGI = bpool.tile([P, T], i32, tag="TAGI")
        nc.vector.tensor_copy(out=TAGI[:, :], in_=TAGF[:, :])
        nc.sync.dma_start(out=tags[:, :], in_=TAGI[:, :])


def make_const_inputs(transitions, start_transitions, end_transitions):
    """Precomputed constant input arrays (replicated across partitions)."""
    import ml_dtypes

    t_T = np.ascontiguousarray(transitions.T.reshape(1, CC)).astype(np.float32)
    t_T = np.repeat(t_T, P, axis=0)
    iota = np.arange(C, dtype=np.float32)
    iota_big = (BIG + iota)[None, :].repeat(C, axis=0).reshape(1, CC)  # [j, i] -> BIG+i
    iota_big = np.repeat(iota_big, P, axis=0).astype(ml_dtypes.bfloat16)
    iota_row = np.repeat(iota[None, :], P, axis=0)
    start_rep = np.repeat(
        start_transitions.astype(np.float32)[None, :], P, axis=0
    )
    end_rep = np.repeat(end_transitions.astype(np.float32)[None, :], P, axis=0)
    return {
        "t_T": np.ascontiguousarray(t_T),
        "iota_big": np.ascontiguousarray(iota_big),
        "iota_row": np.ascontiguousarray(iota_row),
        "start_rep": np.ascontiguousarray(start_rep),
        "end_rep": np.ascontiguousarray(end_rep),
    }


_nc_cache = {}


def kernel(x, start_transitions, end_transitions, transitions):
    T = x.shape[1]
    if T not in _nc_cache:
        _nc_cache[T] = build_viterbi_nc(T)
    nc = _nc_cache[T]
    consts = make_const_inputs(transitions, start_transitions, end_transitions)
    in_maps = []
    for k in range(N_CORES):
        m = {"x": np.ascontiguousarray(x[k * P : (k + 1) * P]).astype(np.float32)}
        m.update(consts)
        in_maps.append(m)
    res = bass_utils.run_bass_kernel_spmd(nc, in_maps, core_ids=list(range(N_CORES)))
    return np.concatenate([r["tags"] for r in res.results], axis=0).astype(np.int32)
